# revision 4
# baseline (speedup 1.0000x reference)
"""Trainium2 Bass kernel for ragged Bahdanau-style additive attention.

Math: scores[t,k] = sum_h va[h]*tanh(Wq[t,h]+Uk[k,h]); softmax over valid keys
k<start; outputs (attentions [B,S,S], contexts [B,S,H]) with ragged zero fill.

Device algorithm (per core = one batch element, data-parallel over B=8):
  tanh(a+b) ~= sum_m beta_m * sin(m*w0*(a+b))
            = sum_m beta_m * [sin_m(a)cos_m(b) + cos_m(a)sin_m(b)]
so the O(T*K*H) tanh becomes R=10 rank-1 products computed by TensorE matmuls
over per-token harmonic features. Features: 2 ACT Sin seeds (args within the
hardware spline's +-3.5 valid window) + Chebyshev-style product ladder on the
VectorEngine in bf16. Softmax numerator/denominator computed on device
(masked exp + ones-matmul row sums); normalization done on host (linear).
"""
import numpy as np
import ml_dtypes

BF16 = ml_dtypes.bfloat16
B, S, H = 8, 512, 768
NH = H // 128  # 6 h-tiles
W0 = 0.44
MULTS = [1, 2, 3, 4, 6]
SSCALE = {1: 1.0, 2: 0.5, 3: 1.0, 4: 0.25, 6: 0.5}
R = 2 * len(MULTS)


def _fit_betas(w0=W0, sigma2=4.0):
    s = np.linspace(-12, 12, 4001)
    w = np.exp(-s ** 2 / sigma2)
    A = np.stack([np.sin(m * w0 * s) for m in MULTS], 1)
    beta, *_ = np.linalg.lstsq(A * np.sqrt(w)[:, None], np.tanh(s) * np.sqrt(w), rcond=None)
    return beta


_BUILD_CACHE = {}


def _build(Tpad, KC):
    from concourse import bacc, mybir
    from concourse import tile

    Kpad = KC * 128
    dt = mybir.dt
    AF = mybir.ActivationFunctionType
    OP = mybir.AluOpType

    nc = bacc.Bacc(None, target_bir_lowering=False, debug=False)
    with tile.TileContext(nc) as tc:
        with tc.tile_pool(name="dram", bufs=1, space="DRAM") as dram:
            d_qt = dram.tile([H, Tpad], dt.bfloat16, kind="ExternalInput", name="qt", uniquify=False)
            d_kt = dram.tile([H, Kpad], dt.bfloat16, kind="ExternalInput", name="kt", uniquify=False)
            d_kin = dram.tile([Kpad, H], dt.bfloat16, kind="ExternalInput", name="kin", uniquify=False)
            d_wat = dram.tile([H, H], dt.bfloat16, kind="ExternalInput", name="wat", uniquify=False)
            d_uat = dram.tile([H, H], dt.bfloat16, kind="ExternalInput", name="uat", uniquify=False)
            d_qb = dram.tile([2, H], dt.float32, kind="ExternalInput", name="qb", uniquify=False)
            d_kb = dram.tile([2, H], dt.float32, kind="ExternalInput", name="kb", uniquify=False)
            d_qmul = dram.tile([R, H], dt.float32, kind="ExternalInput", name="qmul", uniquify=False)
            d_kmask = dram.tile([Kpad, 1], dt.float32, kind="ExternalInput", name="kmask", uniquify=False)
            d_wnum = dram.tile([Kpad, Tpad], dt.bfloat16, kind="ExternalOutput", name="wnum", uniquify=False)
            d_z = dram.tile([1, Tpad], dt.float32, kind="ExternalOutput", name="z", uniquify=False)
            d_ctxt = dram.tile([H, Tpad], dt.bfloat16, kind="ExternalOutput", name="ctxt", uniquify=False)

            with tc.tile_pool(name="sb", bufs=1) as sb:
                # ---- load inputs ----
                qt_sb = sb.tile([128, NH, Tpad], dt.bfloat16)
                kt_sb = sb.tile([128, NH, Kpad], dt.bfloat16)
                kin_sb = sb.tile([128, KC, H], dt.bfloat16)
                wat_sb = sb.tile([128, NH, H], dt.bfloat16)
                uat_sb = sb.tile([128, NH, H], dt.bfloat16)
                qb_sb = sb.tile([128, 2, NH], dt.float32)
                kb_sb = sb.tile([128, 2, NH], dt.float32)
                qmul_sb = sb.tile([128, R, NH], dt.float32)
                kmask_sb = sb.tile([128, KC, 1], dt.float32)
                nc.sync.dma_start(out=wat_sb[:], in_=d_wat.rearrange("(j p) o -> p j o", p=128))
                nc.sync.dma_start(out=uat_sb[:], in_=d_uat.rearrange("(j p) o -> p j o", p=128))
                nc.sync.dma_start(out=qt_sb[:], in_=d_qt.rearrange("(j p) t -> p j t", p=128))
                nc.sync.dma_start(out=kt_sb[:], in_=d_kt.rearrange("(j p) t -> p j t", p=128))
                nc.sync.dma_start(out=kin_sb[:], in_=d_kin.rearrange("(i p) h -> p i h", p=128))
                nc.sync.dma_start(out=qb_sb[:], in_=d_qb.rearrange("r (j p) -> p r j", p=128))
                nc.sync.dma_start(out=kb_sb[:], in_=d_kb.rearrange("r (j p) -> p r j", p=128))
                nc.sync.dma_start(out=qmul_sb[:], in_=d_qmul.rearrange("r (j p) -> p r j", p=128))
                nc.sync.dma_start(out=kmask_sb[:], in_=d_kmask.rearrange("(i p) o -> p i o", p=128))

                # ---- projections + ACT seeds ----
                # seeds for each side: sh = sin(0.5*w0*x + 0.5*w0*bias), s1 = sin(w0*x + w0*bias)
                def side(tag, tok_sb, w_sb, b_sb, Tn):
                    sh = sb.tile([128, NH, Tn], dt.bfloat16, name=f"sh_{tag}", tag="tmp", bufs=4)
                    s1 = sb.tile([128, NH, Tn], dt.bfloat16, name=f"s1_{tag}")
                    for half in range(2):
                        with tc.tile_pool(name=f"pj_{tag}{half}", bufs=1, space="PSUM") as pp:
                            ps = pp.tile([128, 3, 512], dt.float32, name=f"ps_{tag}{half}")
                            for jo_l in range(3):
                                jo = half * 3 + jo_l
                                for jh in range(NH):
                                    nc.tensor.matmul(
                                        ps[:, jo_l, :Tn],
                                        w_sb[:, jh, jo * 128:(jo + 1) * 128],
                                        tok_sb[:, jh, :],
                                        start=(jh == 0), stop=(jh == NH - 1),
                                    )
                                nc.scalar.activation(sh[:, jo, :], ps[:, jo_l, :Tn], AF.Sin,
                                                     bias=b_sb[:, 0, jo:jo + 1], scale=0.5 * W0)
                                nc.scalar.activation(s1[:, jo, :], ps[:, jo_l, :Tn], AF.Sin,
                                                     bias=b_sb[:, 1, jo:jo + 1], scale=W0)
                    return sh, s1

                # ---- DVE harmonic ladder (bf16), per side ----
                def ladder(tag, sh, s1, Tn):
                    f = {}
                    t = lambda nm: sb.tile([128, NH, Tn], dt.bfloat16, name=f"{nm}_{tag}")
                    tmp = lambda nm: sb.tile([128, NH, Tn], dt.bfloat16, name=f"{nm}_{tag}", tag="tmp", bufs=4)
                    c1 = t("c1"); c2 = t("c2")
                    s2 = t("s2"); s3 = t("s3"); c3 = t("c3")
                    s4 = t("s4"); c4 = t("c4"); s6 = t("s6"); c6 = t("c6")
                    TT = nc.vector.tensor_tensor
                    TS = nc.vector.tensor_scalar
                    v1 = tmp("v1")
                    TT(out=v1[:], in0=sh[:], in1=sh[:], op=OP.mult)
                    TS(c1[:], v1[:], -2.0, 1.0, OP.mult, OP.add)
                    v2 = tmp("v2")
                    TT(out=v2[:], in0=s1[:], in1=s1[:], op=OP.mult)
                    TS(c2[:], v2[:], -2.0, 1.0, OP.mult, OP.add)
                    TT(out=s2[:], in0=s1[:], in1=c1[:], op=OP.mult)
                    c2p = tmp("c2p")
                    TS(c2p[:], c2[:], 2.0, 1.0, OP.mult, OP.add)
                    TT(out=s3[:], in0=s1[:], in1=c2p[:], op=OP.mult)
                    c2m = tmp("c2m")
                    TS(c2m[:], c2[:], 2.0, -1.0, OP.mult, OP.add)
                    TT(out=c3[:], in0=c1[:], in1=c2m[:], op=OP.mult)
                    TT(out=s4[:], in0=s2[:], in1=c2[:], op=OP.mult)
                    v4 = tmp("v4")
                    TT(out=v4[:], in0=s2[:], in1=s2[:], op=OP.mult)
                    TS(c4[:], v4[:], -8.0, 1.0, OP.mult, OP.add)
                    TT(out=s6[:], in0=s3[:], in1=c3[:], op=OP.mult)
                    v3 = tmp("v3")
                    TT(out=v3[:], in0=s3[:], in1=s3[:], op=OP.mult)
                    TS(c6[:], v3[:], -2.0, 1.0, OP.mult, OP.add)
                    f['s1'], f['c1'], f['s2'], f['c2'] = s1, c1, s2, c2
                    f['s3'], f['c3'], f['s4'], f['c4'] = s3, c3, s4, c4
                    f['s6'], f['c6'] = s6, c6
                    return f

                # q side: project, seed, ladder, then fold va*amp in place
                sh_q, s1_q = side("q", qt_sb, wat_sb, qb_sb, Tpad)
                fq = ladder("q", sh_q, s1_q, Tpad)
                rank_pairs = []  # (q feature name, k feature name) per rank
                for m in MULTS:
                    rank_pairs.append((f's{m}', f'c{m}'))
                    rank_pairs.append((f'c{m}', f's{m}'))
                for ri, (qnm, knm) in enumerate(rank_pairs):
                    for j in range(NH):
                        nc.vector.tensor_scalar(
                            fq[qnm][:, j, :], fq[qnm][:, j, :],
                            qmul_sb[:, ri, j:j + 1], None, OP.mult)

                # k side
                sh_k, s1_k = side("k", kt_sb, uat_sb, kb_sb, Kpad)
                fk = ladder("k", sh_k, s1_k, Kpad)
                rank_feats = [(fq[qnm], fk[knm]) for qnm, knm in rank_pairs]

                # ---- scores + exp per k-tile ----
                wnum_sb = sb.tile([128, KC, Tpad], dt.bfloat16)
                with tc.tile_pool(name="scp", bufs=2, space="PSUM") as scp:
                    for ki in range(KC):
                        ps = scp.tile([128, 512], dt.float32, name="sc_ps")
                        nmm = len(rank_feats) * NH
                        c = 0
                        for qfeat, kfeat in rank_feats:
                            for jh in range(NH):
                                nc.tensor.matmul(
                                    ps[:, :Tpad],
                                    kfeat[:, jh, ki * 128:(ki + 1) * 128],
                                    qfeat[:, jh, :],
                                    start=(c == 0), stop=(c == nmm - 1),
                                )
                                c += 1
                        nc.scalar.activation(wnum_sb[:, ki, :], ps[:, :Tpad], AF.Exp,
                                             bias=kmask_sb[:, ki, :], scale=1.0)

                # ---- Z row-sums + ctx ----
                ones_sb = sb.tile([128, 1], dt.bfloat16)
                nc.vector.memset(ones_sb[:], 1.0)
                z_sb = sb.tile([1, Tpad], dt.float32)
                ctxt_sb = sb.tile([128, NH, Tpad], dt.bfloat16)
                with tc.tile_pool(name="zcp", bufs=2, space="PSUM") as zcp:
                    zps = zcp.tile([1, 512], dt.float32, name="z_ps", bufs=1)
                    for ki in range(KC):
                        nc.tensor.matmul(zps[:, :Tpad], ones_sb[:], wnum_sb[:, ki, :],
                                         start=(ki == 0), stop=(ki == KC - 1))
                    nc.scalar.copy(z_sb[:], zps[:, :Tpad])
                    for jo in range(NH):
                        cps = zcp.tile([128, 512], dt.float32, name="c_ps")
                        for ki in range(KC):
                            nc.tensor.matmul(cps[:, :Tpad],
                                             kin_sb[:, ki, jo * 128:(jo + 1) * 128],
                                             wnum_sb[:, ki, :],
                                             start=(ki == 0), stop=(ki == KC - 1))
                        nc.scalar.copy(ctxt_sb[:, jo, :], cps[:, :Tpad])

                # ---- outputs ----
                nc.sync.dma_start(out=d_wnum.rearrange("(i p) t -> p i t", p=128), in_=wnum_sb[:])
                nc.sync.dma_start(out=d_z[:], in_=z_sb[:])
                nc.sync.dma_start(out=d_ctxt.rearrange("(j p) t -> p j t", p=128), in_=ctxt_sb[:])
    nc.compile()
    return nc


def get_nc(Tpad, KC):
    key = (Tpad, KC)
    if key not in _BUILD_CACHE:
        _BUILD_CACHE[key] = _build(Tpad, KC)
    return _BUILD_CACHE[key]


def kernel(outputs, mask, Wa_w, Wa_b, Ua_w, Ua_b, Va_w, Va_b, _trace=False):
    from concourse.bass_utils import run_bass_kernel_spmd

    outputs = np.asarray(outputs, np.float32)
    mask = np.asarray(mask)
    Wa_w = np.asarray(Wa_w, np.float32); Wa_b = np.asarray(Wa_b, np.float32)
    Ua_w = np.asarray(Ua_w, np.float32); Ua_b = np.asarray(Ua_b, np.float32)
    va = np.asarray(Va_w, np.float32)[0]

    starts = [int(mask[i, 0]) for i in range(B)]
    nqs = [S - st for st in starts]
    nks = [st for st in starts]
    Tpad = min(512, -(-max(nqs) // 16) * 16)
    KC = -(-max(nks) // 128)
    Kpad = KC * 128

    beta = _fit_betas()
    WaT = np.ascontiguousarray(Wa_w.T).astype(BF16)
    UaT = np.ascontiguousarray(Ua_w.T).astype(BF16)
    qb = np.stack([0.5 * W0 * Wa_b, W0 * Wa_b]).astype(np.float32)
    kb = np.stack([0.5 * W0 * Ua_b, W0 * Ua_b]).astype(np.float32)
    qmul = np.zeros((R, H), np.float32)
    for mi, m in enumerate(MULTS):
        amp = float(beta[mi]) / SSCALE[m]
        qmul[2 * mi] = amp * va
        qmul[2 * mi + 1] = amp * va

    in_maps = []
    for bi in range(B):
        st = starts[bi]; nq = nqs[bi]; nk = nks[bi]
        QT = np.zeros((H, Tpad), np.float32); QT[:, :nq] = outputs[bi, st:, :].T
        KT = np.zeros((H, Kpad), np.float32); KT[:, :nk] = outputs[bi, :nk, :].T
        Kin = np.zeros((Kpad, H), np.float32); Kin[:nk] = outputs[bi, :nk, :]
        kmask = np.zeros((Kpad, 1), np.float32); kmask[nk:] = -1e30
        in_maps.append(dict(
            qt=QT.astype(BF16), kt=KT.astype(BF16), kin=Kin.astype(BF16),
            wat=WaT, uat=UaT, qb=qb, kb=kb, qmul=qmul, kmask=kmask))

    nc = get_nc(Tpad, KC)
    res = run_bass_kernel_spmd(nc, in_maps, core_ids=list(range(B)), trace=_trace)

    attn = np.zeros((B, S, S), np.float32)
    ctx = np.zeros((B, S, H), np.float32)
    for bi in range(B):
        out = res.results[bi]
        nq = nqs[bi]; nk = nks[bi]
        wnum = np.asarray(out["wnum"]).astype(np.float32)
        Z = np.asarray(out["z"]).astype(np.float32)[0]
        ctxT = np.asarray(out["ctxt"]).astype(np.float32)
        attn[bi, :nq, :nk] = (wnum[:nk, :nq] / Z[None, :nq]).T
        ctx[bi, :nq, :] = (ctxT[:, :nq] / Z[None, :nq]).T
    kernel._last_results = res
    return attn, ctx
